# revision 1
# baseline (speedup 1.0000x reference)
"""Channel cross-attention kernel for Trainium2 (8 NeuronCores).

Math (exact restructuring of the reference):
    xf = x.reshape(B, C, N)
    q = wq xf + bq;  k = wk xf + bk;  v = wv xf + bv
    energy = q k^T = wq G wk^T + (wq sx) bk^T + bq (wk sx)^T + N bq bk^T
        where G = xf xf^T (C x C Gram), sx = xf @ 1 (row sums)
    att = softmax(energy / sqrt(N))
    out = att v + xf = (att wv) xf + (att bv) 1^T + xf = M xf + r 1^T + xf

Sharding: 8 cores, core i handles sample b=i//2, spatial half h=i%2.
Each core computes G over the FULL sample (redundantly within the pair, no
cross-core communication) and produces its own spatial half of the output.

Host prep per core: xt = xf[b].T (N, C+2: data, ones, pad), rows ordered
own-spatial-half first. The Gram phase streams all rows; the own half stays
resident in SBUF and is transposed back on-chip for the output phase.
"""

import os
import sys

for _p in ("/opt/trn_rl_repo", "/root/.axon_site/_ro/trn_rl_repo"):
    if os.path.isdir(_p) and _p not in sys.path:
        sys.path.append(_p)

import numpy as np
import ml_dtypes

# ---- problem constants (hardcoded; must match setup_inputs) ----
B, C, W, H = 4, 256, 128, 128
N = W * H            # 16384
HALF = N // 2        # 8192
P = 128              # partitions
NCORES = 8
SQRT_N = float(np.sqrt(N))   # 128.0
XT_COLS_D = C + 2            # xt DRAM row: 256 data cols, ones col, zero pad

# ---- knobs ----
GRAM_DT = os.environ.get("K_GRAM_DT", "f32r")   # f32 | f32r | bf16
MM_DT = os.environ.get("K_MM_DT", "f32r")       # f32 | f32r  (phase C matmuls)

_BUILD_CACHE = {}
LAST_RESULT = None   # BassKernelResults of the most recent run (for test harness)


def _build(gram_dt_name, mm_dt_name):
    import concourse.bacc as bacc
    import concourse.mybir as mybir
    import concourse.tile as tile
    from concourse.bass import MemorySpace
    from concourse.masks import make_identity

    f32 = mybir.dt.float32
    f32r = mybir.dt.float32r
    bf16 = mybir.dt.bfloat16

    # storage dtype of xt in DRAM/SBUF (the Gram matmul dtype)
    xt_store = {"bf16": bf16, "f32r": f32r, "f32": f32}[gram_dt_name]
    # dtype for phase B/C matmul operand tiles
    mmdt = f32r if mm_dt_name == "f32r" else f32

    nc = bacc.Bacc("TRN2", target_bir_lowering=False)

    xt_d = nc.dram_tensor("xt", (N, XT_COLS_D), xt_store, kind="ExternalInput")
    wq_d = nc.dram_tensor("wq", (C, C), f32, kind="ExternalInput")
    bq_d = nc.dram_tensor("bq", (C,), f32, kind="ExternalInput")
    wk_d = nc.dram_tensor("wk", (C, C), f32, kind="ExternalInput")
    bk_d = nc.dram_tensor("bk", (C,), f32, kind="ExternalInput")
    wv_d = nc.dram_tensor("wv", (C, C), f32, kind="ExternalInput")
    bv_d = nc.dram_tensor("bv", (C,), f32, kind="ExternalInput")
    out_d = nc.dram_tensor("out", (C, HALF), f32, kind="ExternalOutput")

    xt_ap = xt_d.ap()
    out_ap = out_d.ap()

    NT = N // P          # 128 n-tiles for the Gram phase
    CH_T = 16            # n-tiles per SBUF tile
    NCHUNK = NT // CH_T  # 8 chunks

    with tile.TileContext(nc) as tc:
        with (
            tc.tile_pool(name="singles", bufs=1) as singles,
            tc.tile_pool(name="work", bufs=2) as work,
        ):
            # ---------- constants ----------
            identity = singles.tile([P, P], f32, tag="ident", name="ident")
            make_identity(nc, identity)
            identity_r = singles.tile([P, P], xt_store, tag="identr", name="identr")
            nc.vector.tensor_copy(out=identity_r, in_=identity)
            # warm the ACT Exp table early so phase B's exp doesn't pay the
            # ~1.3us table load on the critical path
            warm = singles.tile([1, 2], f32, tag="warm", name="warm")
            nc.vector.memset(warm, 0.0)
            nc.scalar.activation(out=warm, in_=warm,
                                 func=mybir.ActivationFunctionType.Exp,
                                 bias=0.0, scale=1.0)

            # weights natural layout (2 row-tiles each)
            wv_sb = []
            wq_sb = []
            wk_sb = []
            for j in range(2):
                t = singles.tile([P, C], f32, tag=f"wv{j}", name=f"wv{j}")
                nc.gpsimd.dma_start(out=t, in_=wv_d.ap()[j * P:(j + 1) * P, :])
                wv_sb.append(t)
                t = singles.tile([P, C], f32, tag=f"wq{j}", name=f"wq{j}")
                nc.gpsimd.dma_start(out=t, in_=wq_d.ap()[j * P:(j + 1) * P, :])
                wq_sb.append(t)
                t = singles.tile([P, C], f32, tag=f"wk{j}", name=f"wk{j}")
                nc.gpsimd.dma_start(out=t, in_=wk_d.ap()[j * P:(j + 1) * P, :])
                wk_sb.append(t)

            bq_row = singles.tile([1, C], f32, tag="bqr", name="bqr")
            nc.gpsimd.dma_start(out=bq_row, in_=bq_d.ap().unsqueeze(0))
            bk_row = singles.tile([1, C], f32, tag="bkr", name="bkr")
            nc.gpsimd.dma_start(out=bk_row, in_=bk_d.ap().unsqueeze(0))
            bkN_row = singles.tile([1, C], f32, tag="bknr", name="bknr")
            nc.gpsimd.dma_start(out=bkN_row, in_=bk_d.ap().unsqueeze(0))
            nc.vector.tensor_scalar_mul(bkN_row, bkN_row, float(N))
            bv_col = []
            for j in range(2):
                t = singles.tile([P, 1], f32, tag=f"bv{j}", name=f"bv{j}")
                nc.gpsimd.dma_start(out=t, in_=bv_d.ap()[j * P:(j + 1) * P].unsqueeze(1))
                bv_col.append(t)

            # transposed weights wqT[e][:, c] = wq[c, e], wkT likewise
            wqT_sb = [singles.tile([P, C], f32, tag=f"wqT{j}", name=f"wqT{j}") for j in range(2)]
            wkT_sb = [singles.tile([P, C], f32, tag=f"wkT{j}", name=f"wkT{j}") for j in range(2)]

            gsb = [singles.tile([P, C + 2], f32, tag=f"gsb{m}", name=f"gsb{m}") for m in range(2)]
            t1sb = [singles.tile([P, C], f32, tag=f"t1sb{m}", name=f"t1sb{m}") for m in range(2)]
            attT_sb = [singles.tile([P, C], f32, tag=f"attT{m}", name=f"attT{m}") for m in range(2)]
            mt_sb = [singles.tile([P, C], mmdt, tag=f"mt{m}", name=f"mt{m}") for m in range(2)]
            r_sb = [singles.tile([P, 1], f32, tag=f"r{m}", name=f"r{m}") for m in range(2)]
            sq_row = singles.tile([1, C], f32, tag="sqr", name="sqr")
            sk_row = singles.tile([1, C], f32, tag="skr", name="skr")
            ssum_sb = [singles.tile([P, 1], f32, tag=f"ssum{m}", name=f"ssum{m}") for m in range(2)]
            rs_sb = [singles.tile([P, 1], f32, tag=f"rs{m}", name=f"rs{m}") for m in range(2)]

            # ---------- phase A: Gram accumulation ----------
            # Own-half chunks (first NCHUNK/2 by host layout) are each
            # Gram-accumulated AND PE-transposed back to natural layout (xn)
            # right after their DMA lands; the chunk buffer then rotates.
            NRES = NCHUNK // 2
            CW = CH_T * P     # chunk width in spatial cols (2048)
            TB = 4            # transposes batched per PSUM bank
            CH_T_S = 8        # n-tiles per streamed (non-resident) SBUF tile
            xn = [[singles.tile([P, CW], xt_store, tag=f"xn{c}_{m}",
                                name=f"xn{c}_{m}") for m in range(2)]
                  for c in range(NRES)]
            with (
                tc.tile_pool(name="psg", bufs=1, space=MemorySpace.PSUM) as psg,
                tc.tile_pool(name="psct", bufs=6, space=MemorySpace.PSUM) as psct,
                tc.tile_pool(name="xtrp", bufs=3) as xtrp,
                tc.tile_pool(name="xtp", bufs=5) as xtp,
            ):
                g_ps = [psg.tile([P, C + 2], f32, tag=f"g{m}", name=f"g{m}") for m in range(2)]
                nt = 0
                for ch in range(NRES):      # own-half chunks
                    xt = xtrp.tile([P, CH_T, XT_COLS_D], xt_store, tag="xtr",
                                   name="xtr")
                    if ch == 0:
                        # split the first chunk so PE starts ~4x sooner
                        QT = CH_T // 4
                        for q in range(4):
                            nc.sync.dma_start(
                                out=xt[:, q * QT:(q + 1) * QT, :],
                                in_=xt_ap[q * QT * P:(q + 1) * QT * P, :]
                                    .rearrange("(t p) c -> p t c", p=P),
                            )
                    else:
                        nc.sync.dma_start(
                            out=xt,
                            in_=xt_ap[ch * CH_T * P:(ch + 1) * CH_T * P, :]
                                .rearrange("(t p) c -> p t c", p=P),
                        )
                    for t in range(CH_T):
                        for m in range(2):
                            nc.tensor.matmul(
                                g_ps[m][:, 0:C + 2],
                                xt[:, t, m * P:(m + 1) * P],
                                xt[:, t, 0:C + 2],
                                start=(nt == 0), stop=False,
                            )
                        nt += 1
                    # transpose back to natural layout while the next chunk
                    # streams in
                    for m in range(2):
                        for tb in range(CH_T // TB):
                            tps = psct.tile([P, TB * P], xt_store, tag="tps",
                                            name="tps")
                            for k in range(TB):
                                t = tb * TB + k
                                nc.tensor.transpose(
                                    tps[:, k * P:(k + 1) * P],
                                    xt[:, t, m * P:(m + 1) * P],
                                    identity_r)
                            nc.vector.tensor_copy(
                                out=xn[ch][m][:, tb * TB * P:(tb + 1) * TB * P],
                                in_=tps)

                # streamed other-half chunks; weight transposes fill the
                # PE idle time (stream DMA > stream PE work)
                wjobs = [(wsrc, wdst, jj, ii)
                         for (wsrc, wdst) in ((wq_sb, wqT_sb), (wk_sb, wkT_sb))
                         for jj in range(2) for ii in range(2)]
                nstream = (NCHUNK - NRES) * CH_T // CH_T_S
                for sch in range(nstream):
                    xt = xtp.tile([P, CH_T_S, XT_COLS_D], xt_store, tag="xt",
                                  name="xt")
                    row0 = NRES * CH_T * P + sch * CH_T_S * P
                    nc.sync.dma_start(
                        out=xt,
                        in_=xt_ap[row0:row0 + CH_T_S * P, :]
                            .rearrange("(t p) c -> p t c", p=P),
                    )
                    for t in range(CH_T_S):
                        for m in range(2):
                            nc.tensor.matmul(
                                g_ps[m][:, 0:C + 2],
                                xt[:, t, m * P:(m + 1) * P],
                                xt[:, t, 0:C + 2],
                                start=False, stop=(nt == NT - 1),
                            )
                        nt += 1
                    if sch < len(wjobs):
                        wsrc, wdst, jj, ii = wjobs[sch]
                        ps = psct.tile([P, P], f32, tag="tps", name="wt")
                        nc.tensor.transpose(
                            ps, wsrc[ii][:, jj * P:(jj + 1) * P], identity)
                        nc.vector.tensor_copy(
                            out=wdst[jj][:, ii * P:(ii + 1) * P], in_=ps)

                # copy G (+ sx column) to SBUF
                for m in range(2):
                    nc.vector.tensor_copy(out=gsb[m], in_=g_ps[m])

            # ---------- phase B: energy^T, exp, M, r ----------
            # Everything is computed directly in the TRANSPOSED (d, c)
            # orientation (G is symmetric), so no PE<->DVE transpose
            # ping-pong. exp is taken without max-subtraction (energies
            # here are |e|/sqrt(N) < ~50, exp < 1e20, far from fp32
            # overflow); the 1/rowsum normalization is folded into the
            # phase-C output scale.
            with tc.tile_pool(name="psb", bufs=1, space=MemorySpace.PSUM) as psb:
                # T1q[a, c] = (G wq^T)[a, c]
                for a in range(2):
                    ps = psb.tile([P, C], f32, tag="tmp", name="tmp", bufs=4)
                    for e in range(2):
                        nc.tensor.matmul(
                            ps,
                            gsb[e][:, a * P:(a + 1) * P],
                            wqT_sb[e][:, 0:C],
                            start=(e == 0), stop=(e == 1),
                        )
                    nc.vector.tensor_copy(out=t1sb[a], in_=ps)

                # sq' = wq sx (row), sk' = wk sx (row)
                ps = psb.tile([1, C], f32, tag="tmp", name="tmp", bufs=4)
                for e in range(2):
                    nc.tensor.matmul(
                        ps, gsb[e][:, C:C + 1], wqT_sb[e][:, 0:C],
                        start=(e == 0), stop=(e == 1))
                nc.vector.tensor_copy(out=sq_row, in_=ps)
                ps = psb.tile([1, C], f32, tag="tmp", name="tmp", bufs=4)
                for e in range(2):
                    nc.tensor.matmul(
                        ps, gsb[e][:, C:C + 1], wkT_sb[e][:, 0:C],
                        start=(e == 0), stop=(e == 1))
                nc.vector.tensor_copy(out=sk_row, in_=ps)

                # energyT (raw, unscaled) per d-tile:
                # energyT[d, c] = (wk G wq^T)[d, c] + bk[d] sq'[c]
                #                 + sk'[d] bq[c] + N bk[d] bq[c]
                eT_ps = [psb.tile([P, C], f32, tag=f"eps{dt}", name=f"eps{dt}")
                         for dt in range(2)]
                for dt in range(2):
                    for a in range(2):
                        nc.tensor.matmul(
                            eT_ps[dt],
                            wkT_sb[a][:, dt * P:(dt + 1) * P],
                            t1sb[a][:, 0:C],
                            start=(a == 0), stop=False,
                        )
                    ds_ = (dt * P, (dt + 1) * P)
                    nc.tensor.matmul(eT_ps[dt], bk_row[:, ds_[0]:ds_[1]],
                                     sq_row[:, 0:C], start=False, stop=False)
                    nc.tensor.matmul(eT_ps[dt], sk_row[:, ds_[0]:ds_[1]],
                                     bq_row[:, 0:C], start=False, stop=False)
                    nc.tensor.matmul(eT_ps[dt], bkN_row[:, ds_[0]:ds_[1]],
                                     bq_row[:, 0:C], start=False, stop=True)
                    # attT (unnormalized): exp(energyT / sqrt(N))
                    nc.scalar.activation(
                        out=attT_sb[dt], in_=eT_ps[dt],
                        func=mybir.ActivationFunctionType.Exp,
                        bias=0.0, scale=1.0 / SQRT_N)

                ones_col = singles.tile([P, 1], f32, tag="ones", name="ones")
                nc.vector.memset(ones_col, 1.0)

                # row sums: ssum[c] = sum_d attT[d, c] (column via matmul)
                for ct in range(2):
                    ps = psb.tile([P, 1], f32, tag="rps", name="rps", bufs=2)
                    for d in range(2):
                        nc.tensor.matmul(
                            ps, attT_sb[d][:, ct * P:(ct + 1) * P], ones_col,
                            start=(d == 0), stop=(d == 1))
                    nc.vector.tensor_copy(out=ssum_sb[ct], in_=ps)
                    nc.vector.reciprocal(out=rs_sb[ct], in_=ssum_sb[ct])

                # MT[e][:, c] = M~[c, e] = sum_d att~[c, d] wv[d, e]
                # (+ diag(ssum) so phase C's rs scale yields M x + x)
                for e in range(2):
                    ps = psb.tile([P, C], f32, tag="tmp", name="tmp", bufs=4)
                    for d in range(2):
                        nc.tensor.matmul(
                            ps,
                            wv_sb[d][:, e * P:(e + 1) * P],
                            attT_sb[d][:, 0:C],
                            start=(d == 0), stop=(d == 1),
                        )
                    nc.vector.tensor_copy(out=mt_sb[e], in_=ps)
                    dg = work.tile([P, P], f32, tag="dg", name="dg")
                    nc.vector.tensor_scalar_mul(dg, identity, ssum_sb[e])
                    nc.vector.tensor_add(
                        mt_sb[e][:, e * P:(e + 1) * P],
                        mt_sb[e][:, e * P:(e + 1) * P],
                        dg)

                # r[c] = rs[c] * sum_d att~[c, d] bv[d]
                for ct in range(2):
                    ps = psb.tile([P, 1], f32, tag="rps", name="rps", bufs=2)
                    for d in range(2):
                        nc.tensor.matmul(
                            ps, attT_sb[d][:, ct * P:(ct + 1) * P], bv_col[d],
                            start=(d == 0), stop=(d == 1))
                    nc.vector.tensor_copy(out=r_sb[ct], in_=ps)
                    nc.vector.tensor_mul(r_sb[ct], r_sb[ct], rs_sb[ct])

            # ---------- phase C: out = (M + I) x + r ----------
            MMW = 512         # matmul free width (one PSUM bank of f32)
            with (
                tc.tile_pool(name="psc", bufs=2, space=MemorySpace.PSUM) as psc,
                tc.tile_pool(name="outp", bufs=2) as outp,
            ):
                for ch in range(NRES):
                    for ct in range(2):
                        ot = outp.tile([P, CW], f32, tag=f"ot{ct}", name=f"ot{ct}")
                        for s in range(CW // MMW):
                            sl = slice(s * MMW, (s + 1) * MMW)
                            ps = psc.tile([P, MMW], f32, tag=f"ops{ct}",
                                          name=f"ops{ct}")
                            for e in range(2):
                                nc.tensor.matmul(
                                    ps,
                                    mt_sb[e][:, ct * P:(ct + 1) * P],
                                    xn[ch][e][:, sl],
                                    start=(e == 0), stop=(e == 1),
                                )
                            # ot = rs*psum + r (per-partition scale+bias)
                            nc.scalar.activation(
                                out=ot[:, sl], in_=ps,
                                func=mybir.ActivationFunctionType.Identity,
                                bias=r_sb[ct], scale=rs_sb[ct])
                            if ch == 0:
                                # first chunk: per-sub-block DMA so the
                                # output stream starts immediately
                                nc.sync.dma_start(
                                    out=out_ap[ct * P:(ct + 1) * P,
                                               ch * CW + s * MMW:
                                               ch * CW + (s + 1) * MMW],
                                    in_=ot[:, sl])
                        if ch > 0:
                            nc.sync.dma_start(
                                out=out_ap[ct * P:(ct + 1) * P,
                                           ch * CW:(ch + 1) * CW],
                                in_=ot)

    nc.compile()
    return nc


def _get_nc():
    key = (GRAM_DT, MM_DT)
    if key not in _BUILD_CACHE:
        _BUILD_CACHE[key] = _build(*key)
    return _BUILD_CACHE[key]


def kernel(x, wq, bq, wk, bk, wv, bv):
    global LAST_RESULT
    from concourse.bass_utils import run_bass_kernel_spmd

    nc = _get_nc()

    x = np.ascontiguousarray(np.asarray(x, dtype=np.float32))
    xf = x.reshape(B, C, N)
    wq = np.ascontiguousarray(np.asarray(wq, dtype=np.float32))
    wk = np.ascontiguousarray(np.asarray(wk, dtype=np.float32))
    wv = np.ascontiguousarray(np.asarray(wv, dtype=np.float32))
    bq = np.ascontiguousarray(np.asarray(bq, dtype=np.float32))
    bk = np.ascontiguousarray(np.asarray(bk, dtype=np.float32))
    bv = np.ascontiguousarray(np.asarray(bv, dtype=np.float32))

    in_maps = _make_in_maps(xf, wq, bq, wk, bk, wv, bv)

    res = run_bass_kernel_spmd(nc, in_maps, core_ids=list(range(NCORES)))
    LAST_RESULT = res

    out = np.empty((B, C, N), dtype=np.float32)
    for i in range(NCORES):
        b, h = i // 2, i % 2
        out[b, :, h * HALF:(h + 1) * HALF] = res.results[i]["out"]
    return out.reshape(B, C, W, H)


# ---------------------------------------------------------------------------
# Dev-loop helpers (not used by the grading path)
# ---------------------------------------------------------------------------

def timeline_ns():
    """Cost-model simulated duration of one core's program (ns)."""
    from concourse.timeline_sim import TimelineSim
    nc = _get_nc()
    ts = TimelineSim(nc)
    return ts.simulate()


def bench(in_maps=None, iters=8):
    """Steady-state wall time per execution of the 8-core sharded NEFF.

    Mirrors bass2jax.run_bass_via_pjrt's multi-core path, without donation,
    jitted once, inputs staged on device. Returns (times_s, results_list).
    """
    import jax
    import numpy as _np
    from jax.sharding import Mesh, PartitionSpec
    from jax.experimental.shard_map import shard_map
    import concourse.bass2jax as b2j
    import concourse.mybir as mybir
    import time as _time

    nc = _get_nc()
    b2j.install_neuronx_cc_hook()

    partition_name = (nc.partition_id_tensor.name
                      if nc.partition_id_tensor else None)
    in_names, out_names, out_avals, zero_outs = [], [], [], []
    for alloc in nc.m.functions[0].allocations:
        if not isinstance(alloc, mybir.MemoryLocationSet):
            continue
        name = alloc.memorylocations[0].name
        if alloc.kind == "ExternalInput":
            if name != partition_name:
                in_names.append(name)
        elif alloc.kind == "ExternalOutput":
            out_names.append(name)
            shape = tuple(alloc.tensor_shape)
            dtype = mybir.dt.np(alloc.dtype)
            out_avals.append(jax.core.ShapedArray(shape, dtype))
            zero_outs.append(_np.zeros(shape, dtype))
    n_params = len(in_names)
    all_names = in_names + out_names
    if partition_name is not None:
        all_names = all_names + [partition_name]

    def _body(*args):
        operands = list(args)
        if partition_name is not None:
            operands.append(b2j.partition_id_tensor())
        outs = b2j._bass_exec_p.bind(
            *operands,
            out_avals=tuple(out_avals),
            in_names=tuple(all_names),
            out_names=tuple(out_names),
            lowering_input_output_aliases=(),
            sim_require_finite=True,
            sim_require_nnan=True,
            nc=nc,
        )
        return tuple(outs)

    devices = jax.devices()[:NCORES]
    mesh = Mesh(_np.asarray(devices), ("core",))
    nin = n_params + len(out_names)
    sharded = jax.jit(
        shard_map(_body, mesh=mesh,
                  in_specs=(PartitionSpec("core"),) * nin,
                  out_specs=(PartitionSpec("core"),) * len(out_names),
                  check_rep=False),
        keep_unused=True,
    )

    if in_maps is None:
        rng = _np.random.default_rng(0)
        x = rng.standard_normal((B, C, W, H), dtype=_np.float32)
        s = 1.0 / _np.sqrt(C)
        ws = {k: (rng.standard_normal((C, C), dtype=_np.float32) * s)
              for k in ("wq", "wk", "wv")}
        bs = {k: (rng.standard_normal((C,), dtype=_np.float32) * s)
              for k in ("bq", "bk", "bv")}
        in_maps = _make_in_maps(x.reshape(B, C, N), ws["wq"], bs["bq"],
                                ws["wk"], bs["bk"], ws["wv"], bs["bv"])

    concat_in = [_np.concatenate([_np.asarray(in_maps[c][nm])
                                  for c in range(NCORES)], axis=0)
                 for nm in in_names]
    concat_zeros = [_np.zeros((NCORES * z.shape[0], *z.shape[1:]), z.dtype)
                    for z in zero_outs]
    dev_args = [jax.device_put(a) for a in concat_in + concat_zeros]
    for a in dev_args:
        a.block_until_ready()

    # warmup (compiles)
    outs = sharded(*dev_args)
    jax.block_until_ready(outs)

    times = []
    for _ in range(iters):
        t0 = _time.perf_counter()
        outs = sharded(*dev_args)
        jax.block_until_ready(outs)
        times.append(_time.perf_counter() - t0)
    return times, outs


def _make_in_maps(xf, wq, bq, wk, bk, wv, bv):
    xt_np_dt = ml_dtypes.bfloat16 if GRAM_DT == "bf16" else np.float32
    ones_pad = np.zeros((N, 2), dtype=np.float32)
    ones_pad[:, 0] = 1.0
    in_maps = []
    for i in range(NCORES):
        b, h = i // 2, i % 2
        xTb = np.concatenate([xf[b].T, ones_pad], axis=1).astype(xt_np_dt)
        # own spatial half first: the kernel keeps the first NCHUNK/2 chunks
        # resident and derives its output columns from them
        xt = np.concatenate([xTb[h * HALF:(h + 1) * HALF],
                             xTb[(1 - h) * HALF:(2 - h) * HALF]], axis=0)
        in_maps.append({
            "xt": xt,
            "wq": wq, "bq": bq, "wk": wk, "bk": bk, "wv": wv, "bv": bv,
        })
    return in_maps



# revision 9
# speedup vs baseline: 1.6207x; 1.6207x over previous
"""Channel cross-attention kernel for Trainium2 (8 NeuronCores).

Math (exact restructuring of the reference):
    xf = x.reshape(B, C, N)
    q = wq xf + bq;  k = wk xf + bk;  v = wv xf + bv
    energy = q k^T = wq G wk^T + (wq sx) bk^T + bq (wk sx)^T + N bq bk^T
        where G = xf xf^T (C x C Gram), sx = xf @ 1 (row sums)
    att = softmax(energy / sqrt(N))
    out = att v + xf = (att wv) xf + (att bv) 1^T + xf = M xf + r 1^T + xf

Sharding: 8 cores, core i handles sample b=i//2, spatial half h=i%2.
Each core computes G over the FULL sample (redundantly within the pair, no
cross-core communication) and produces its own spatial half of the output.

v2 design (all-bf16 data plane):
  - xt (N, 258) bf16: transposed sample (own half first) streamed once for
    the Gram.  Triangle trick: G is symmetric, so the second c-tile only
    computes cols 128..256 (129 wide); the missing 128x128 block is
    reconstructed with one PE transpose.
  - xn (C, HALF) bf16: natural-layout own half.  The last K_STREAM chunks
    are streamed from DRAM; the first 4-K_STREAM chunks are PE-transposed
    from the xt stream (PSUM->SBUF copies alternate DVE/ACT).
  - Phase B runs entirely in f32r (1 cycle/row on PE vs 4 for f32), with
    host-pretransposed wqT/wkT.  exp is taken without max-subtraction
    (|energy|/sqrt(N) < ~50, far from fp32 overflow); the 1/rowsum scale
    is folded into the phase-C output op.
  - Phase C: out = rs*(M~ + diag(ssum)) xn + r, matmuls in bf16, the
    scale+bias PSUM->SBUF op alternates ACT/DVE, output written bf16 and
    upcast to f32 on the host.
"""

import os
import sys

for _p in ("/opt/trn_rl_repo", "/root/.axon_site/_ro/trn_rl_repo"):
    if os.path.isdir(_p) and _p not in sys.path:
        sys.path.append(_p)

import numpy as np
import ml_dtypes

# ---- problem constants (hardcoded; must match setup_inputs) ----
B, C, W, H = 4, 256, 128, 128
N = W * H            # 16384
HALF = N // 2        # 8192
P = 128              # partitions
NCORES = 8
SQRT_N = float(np.sqrt(N))   # 128.0
XT_COLS = C + 2              # xt row: 256 data cols, ones col, zero pad (516B)

# ---- knobs ----
K_STREAM = int(os.environ.get("K_STREAM", "1"))   # own chunks streamed as xn

_BUILD_CACHE = {}
LAST_RESULT = None   # BassKernelResults of the most recent run (for test harness)

# kept for test harness printout compat
GRAM_DT = "bf16"
MM_DT = "bf16"


def _build(k_stream):
    import concourse.bacc as bacc
    import concourse.mybir as mybir
    import concourse.tile as tile
    from concourse.bass import MemorySpace
    from concourse.masks import make_identity

    f32 = mybir.dt.float32
    f32r = mybir.dt.float32r
    bf16 = mybir.dt.bfloat16

    nc = bacc.Bacc("TRN2", target_bir_lowering=False)

    xt_d = nc.dram_tensor("xt", (N, XT_COLS), bf16, kind="ExternalInput")
    xn_d = nc.dram_tensor("xn", (C, HALF), bf16, kind="ExternalInput")
    wqt_d = nc.dram_tensor("wqt", (C, C), f32r, kind="ExternalInput")
    wkt_d = nc.dram_tensor("wkt", (C, C), f32r, kind="ExternalInput")
    wv_d = nc.dram_tensor("wv", (C, C), f32r, kind="ExternalInput")
    bq_d = nc.dram_tensor("bq", (C,), f32r, kind="ExternalInput")
    bk_d = nc.dram_tensor("bk", (C,), f32r, kind="ExternalInput")
    bv_d = nc.dram_tensor("bv", (C,), f32, kind="ExternalInput")
    out_d = nc.dram_tensor("out", (C, HALF), bf16, kind="ExternalOutput")

    xt_ap = xt_d.ap()
    xn_ap = xn_d.ap()
    out_ap = out_d.ap()

    NT = N // P          # 128 n-tiles for the Gram phase
    CH_T = 16            # n-tiles per own-half chunk
    NRES = 4             # own-half chunks
    CW = CH_T * P        # chunk width in spatial cols (2048)
    TB = 4               # transposes batched per PSUM bank
    CH_T_S = 8           # n-tiles per streamed other-half chunk
    NTRANS = NRES - k_stream   # own chunks transposed on-chip

    with tile.TileContext(nc) as tc:
        with (
            tc.tile_pool(name="singles", bufs=1) as singles,
            tc.tile_pool(name="work", bufs=2) as work,
        ):
            # ---------- constants ----------
            identity = singles.tile([P, P], f32, tag="ident", name="ident")
            make_identity(nc, identity)
            identity_b = singles.tile([P, P], bf16, tag="identb", name="identb")
            nc.vector.tensor_copy(out=identity_b, in_=identity)
            identity_r = singles.tile([P, P], f32r, tag="identr", name="identr")
            nc.vector.tensor_copy(out=identity_r, in_=identity)
            # warm the ACT Exp table early so phase B's exp doesn't pay the
            # ~1.3us table load on the critical path
            warm = singles.tile([1, 2], f32, tag="warm", name="warm")
            nc.vector.memset(warm, 0.0)
            nc.scalar.activation(out=warm, in_=warm,
                                 func=mybir.ActivationFunctionType.Exp,
                                 bias=0.0, scale=1.0)
            ones_col = singles.tile([P, 1], f32, tag="ones", name="ones")
            nc.vector.memset(ones_col, 1.0)

            # ---------- weights / biases (f32r tiles) ----------
            wqT_sb = []
            wkT_sb = []
            wv_sb = []
            for j in range(2):
                t = singles.tile([P, C], f32r, tag=f"wqT{j}", name=f"wqT{j}")
                wqT_sb.append(t)
                t = singles.tile([P, C], f32r, tag=f"wkT{j}", name=f"wkT{j}")
                wkT_sb.append(t)
                t = singles.tile([P, C], f32r, tag=f"wv{j}", name=f"wv{j}")
                wv_sb.append(t)
            bq_row = singles.tile([1, C], f32r, tag="bqr", name="bqr")
            bk_row = singles.tile([1, C], f32r, tag="bkr", name="bkr")
            bkN_row = singles.tile([1, C], f32r, tag="bknr", name="bknr")
            bv_col = [singles.tile([P, 1], f32, tag=f"bv{j}", name=f"bv{j}")
                      for j in range(2)]

            def load_weights():
                for j in range(2):
                    nc.sync.dma_start(out=wqT_sb[j],
                                      in_=wqt_d.ap()[j * P:(j + 1) * P, :])
                    nc.sync.dma_start(out=wkT_sb[j],
                                      in_=wkt_d.ap()[j * P:(j + 1) * P, :])
                    nc.sync.dma_start(out=wv_sb[j],
                                      in_=wv_d.ap()[j * P:(j + 1) * P, :])
                    nc.sync.dma_start(out=bv_col[j],
                                      in_=bv_d.ap()[j * P:(j + 1) * P].unsqueeze(1))
                nc.sync.dma_start(out=bq_row, in_=bq_d.ap().unsqueeze(0))
                nc.sync.dma_start(out=bk_row, in_=bk_d.ap().unsqueeze(0))
                nc.vector.tensor_scalar_mul(bkN_row, bk_row, float(N))

            # ---------- phase A state ----------
            # xn: natural-layout own half, resident (chunk, m) tiles
            xn = [[singles.tile([P, CW], bf16, tag=f"xn{c}_{m}",
                                name=f"xn{c}_{m}") for m in range(2)]
                  for c in range(NRES)]
            gsb = [singles.tile([P, XT_COLS], f32r, tag=f"gsb{m}",
                                name=f"gsb{m}") for m in range(2)]
            t1sb = [singles.tile([P, C], f32r, tag=f"t1sb{m}", name=f"t1sb{m}")
                    for m in range(2)]
            attT_sb = [singles.tile([P, C], f32r, tag=f"attT{m}",
                                    name=f"attT{m}") for m in range(2)]
            mt_sb = [singles.tile([P, C], bf16, tag=f"mt{m}", name=f"mt{m}")
                     for m in range(2)]
            r_sb = [singles.tile([P, 1], f32, tag=f"r{m}", name=f"r{m}")
                    for m in range(2)]
            sq_row = singles.tile([1, C], f32r, tag="sqr", name="sqr")
            sk_row = singles.tile([1, C], f32r, tag="skr", name="skr")
            ssum_sb = [singles.tile([P, 1], f32, tag=f"ssum{m}",
                                    name=f"ssum{m}") for m in range(2)]
            rs_sb = [singles.tile([P, 1], f32, tag=f"rs{m}", name=f"rs{m}")
                     for m in range(2)]

            # ---------- phase A: Gram accumulation (triangle) ----------
            # g0[c, d] = sum_n x_c x_d  (c in tile0, d in 0..255 + sx col)
            # g1[c, d] for c in tile1 only covers d in 128..255 + sx col;
            # the lower-left block comes from transposing g0's upper-right.
            ncopy = 0
            with (
                tc.tile_pool(name="psg", bufs=1, space=MemorySpace.PSUM) as psg,
                tc.tile_pool(name="psct", bufs=4, space=MemorySpace.PSUM) as psct,
                tc.tile_pool(name="xtrp", bufs=3) as xtrp,
                tc.tile_pool(name="xtp", bufs=5) as xtp,
            ):
                g0_ps = psg.tile([P, C + 1], f32, tag="g0", name="g0")
                g1_ps = psg.tile([P, P + 1], f32, tag="g1", name="g1")
                nt = 0

                def gram_tile(xt, t):
                    nonlocal nt
                    nc.tensor.matmul(
                        g0_ps, xt[:, t, 0:P], xt[:, t, 0:C + 1],
                        start=(nt == 0), stop=(nt == NT - 1))
                    nc.tensor.matmul(
                        g1_ps, xt[:, t, P:C], xt[:, t, P:C + 1],
                        start=(nt == 0), stop=(nt == NT - 1))
                    nt += 1

                for ch in range(NRES):      # own-half chunks
                    xt = xtrp.tile([P, CH_T, XT_COLS], bf16, tag="xtr",
                                   name="xtr")
                    if ch == 0:
                        # split the first chunk so PE starts ~4x sooner
                        QT = CH_T // 4
                        for q in range(4):
                            nc.sync.dma_start(
                                out=xt[:, q * QT:(q + 1) * QT, :],
                                in_=xt_ap[q * QT * P:(q + 1) * QT * P, :]
                                    .rearrange("(t p) c -> p t c", p=P),
                            )
                    else:
                        nc.sync.dma_start(
                            out=xt,
                            in_=xt_ap[ch * CH_T * P:(ch + 1) * CH_T * P, :]
                                .rearrange("(t p) c -> p t c", p=P),
                        )
                    if ch == 1:
                        load_weights()
                    for t in range(CH_T):
                        gram_tile(xt, t)
                    if ch < NTRANS:
                        # transpose back to natural layout while the next
                        # chunk streams in; copies alternate DVE/ACT
                        for m in range(2):
                            for tb in range(CH_T // TB):
                                tps = psct.tile([P, TB * P], bf16, tag="tps",
                                                name="tps")
                                for k in range(TB):
                                    t = tb * TB + k
                                    nc.tensor.transpose(
                                        tps[:, k * P:(k + 1) * P],
                                        xt[:, t, m * P:(m + 1) * P],
                                        identity_b)
                                dst = xn[ch][m][:, tb * TB * P:(tb + 1) * TB * P]
                                if ncopy % 2 == 0:
                                    nc.vector.tensor_copy(out=dst, in_=tps)
                                else:
                                    nc.scalar.copy(out=dst, in_=tps)
                                ncopy += 1

                # streamed other-half chunks
                nstream = (NT - NRES * CH_T) // CH_T_S
                for sch in range(nstream):
                    xt = xtp.tile([P, CH_T_S, XT_COLS], bf16, tag="xt",
                                  name="xt")
                    row0 = NRES * CH_T * P + sch * CH_T_S * P
                    nc.sync.dma_start(
                        out=xt,
                        in_=xt_ap[row0:row0 + CH_T_S * P, :]
                            .rearrange("(t p) c -> p t c", p=P),
                    )
                    for t in range(CH_T_S):
                        gram_tile(xt, t)

                # xn stream for the last k_stream own chunks (lands during
                # late phase A / phase B; consumed at the end of phase C)
                for ch in range(NTRANS, NRES):
                    for m in range(2):
                        nc.sync.dma_start(
                            out=xn[ch][m],
                            in_=xn_ap[m * P:(m + 1) * P,
                                      ch * CW:(ch + 1) * CW],
                        )

                # assemble G in SBUF:
                # gsb0 = g0 (257 cols); gsb1[:, 0:128] = g0[:, 128:256]^T,
                # gsb1[:, 128:257] = g1
                nc.vector.tensor_copy(out=gsb[0][:, 0:C + 1], in_=g0_ps)
                nc.vector.tensor_copy(out=gsb[1][:, P:C + 1], in_=g1_ps)

            # ---------- phase B: energy^T, exp, M, r (all f32r) ----------
            with tc.tile_pool(name="psb", bufs=1, space=MemorySpace.PSUM) as psb:
                g10_ps = psb.tile([P, P], f32r, tag="g10", name="g10")
                nc.tensor.transpose(g10_ps, gsb[0][:, P:C], identity_r)
                nc.vector.tensor_copy(out=gsb[1][:, 0:P], in_=g10_ps)
                # T1[a, c] = (G wq^T)[a, c]
                for a in range(2):
                    ps = psb.tile([P, C], f32, tag="tmp", name="tmp", bufs=2)
                    for e in range(2):
                        nc.tensor.matmul(
                            ps,
                            gsb[e][:, a * P:(a + 1) * P],
                            wqT_sb[e][:, 0:C],
                            start=(e == 0), stop=(e == 1),
                        )
                    nc.vector.tensor_copy(out=t1sb[a], in_=ps)

                # sq' = wq sx (row), sk' = wk sx (row)
                ps = psb.tile([1, C], f32, tag="srow", name="srow", bufs=1)
                for e in range(2):
                    nc.tensor.matmul(
                        ps, gsb[e][:, C:C + 1], wqT_sb[e][:, 0:C],
                        start=(e == 0), stop=(e == 1))
                nc.vector.tensor_copy(out=sq_row, in_=ps)
                ps = psb.tile([1, C], f32, tag="srow", name="srow", bufs=1)
                for e in range(2):
                    nc.tensor.matmul(
                        ps, gsb[e][:, C:C + 1], wkT_sb[e][:, 0:C],
                        start=(e == 0), stop=(e == 1))
                nc.vector.tensor_copy(out=sk_row, in_=ps)

                # energyT (raw, unscaled) per d-tile:
                # energyT[d, c] = (wk G wq^T)[d, c] + bk[d] sq'[c]
                #                 + sk'[d] bq[c] + N bk[d] bq[c]
                for dt in range(2):
                    eps = psb.tile([P, C], f32, tag=f"eps{dt}", name=f"eps{dt}")
                    for a in range(2):
                        nc.tensor.matmul(
                            eps,
                            wkT_sb[a][:, dt * P:(dt + 1) * P],
                            t1sb[a][:, 0:C],
                            start=(a == 0), stop=False,
                        )
                    ds_ = (dt * P, (dt + 1) * P)
                    nc.tensor.matmul(eps, bk_row[:, ds_[0]:ds_[1]],
                                     sq_row[:, 0:C], start=False, stop=False)
                    nc.tensor.matmul(eps, sk_row[:, ds_[0]:ds_[1]],
                                     bq_row[:, 0:C], start=False, stop=False)
                    nc.tensor.matmul(eps, bkN_row[:, ds_[0]:ds_[1]],
                                     bq_row[:, 0:C], start=False, stop=True)
                    # attT (unnormalized): exp(energyT / sqrt(N))
                    nc.scalar.activation(
                        out=attT_sb[dt], in_=eps,
                        func=mybir.ActivationFunctionType.Exp,
                        bias=0.0, scale=1.0 / SQRT_N)

                # row sums: ssum[c] = sum_d attT[d, c] (column via matmul)
                for ct in range(2):
                    ps = psb.tile([P, 1], f32, tag="rps", name="rps", bufs=2)
                    for d in range(2):
                        nc.tensor.matmul(
                            ps,
                            attT_sb[d].bitcast(f32)[:, ct * P:(ct + 1) * P],
                            ones_col,
                            start=(d == 0), stop=(d == 1))
                    nc.vector.tensor_copy(out=ssum_sb[ct], in_=ps)
                    nc.vector.reciprocal(out=rs_sb[ct], in_=ssum_sb[ct])

                # MT[e][:, c] = M~[c, e] = sum_d att~[c, d] wv[d, e]
                # (+ diag(ssum) so phase C's rs scale yields M x + x)
                for e in range(2):
                    ps = psb.tile([P, C], f32, tag="tmp", name="tmp", bufs=2)
                    for d in range(2):
                        nc.tensor.matmul(
                            ps,
                            wv_sb[d][:, e * P:(e + 1) * P],
                            attT_sb[d][:, 0:C],
                            start=(d == 0), stop=(d == 1),
                        )
                    nc.vector.tensor_copy(out=mt_sb[e], in_=ps)
                    dg = work.tile([P, P], bf16, tag="dg", name="dg")
                    nc.vector.tensor_scalar_mul(dg, identity_b, ssum_sb[e])
                    nc.vector.tensor_add(
                        mt_sb[e][:, e * P:(e + 1) * P],
                        mt_sb[e][:, e * P:(e + 1) * P],
                        dg)

                # r[c] = rs[c] * sum_d att~[c, d] bv[d]
                for ct in range(2):
                    ps = psb.tile([P, 1], f32, tag="rps", name="rps", bufs=2)
                    for d in range(2):
                        nc.tensor.matmul(
                            ps,
                            attT_sb[d].bitcast(f32)[:, ct * P:(ct + 1) * P],
                            bv_col[d],
                            start=(d == 0), stop=(d == 1))
                    nc.vector.tensor_copy(out=r_sb[ct], in_=ps)
                    nc.vector.tensor_mul(r_sb[ct], r_sb[ct], rs_sb[ct])

            # ---------- phase C: out = rs*(M~ + diag(ssum)) xn + r ----------
            MMW = 512         # matmul free width (one PSUM bank of f32)
            nsb = 0
            with (
                tc.tile_pool(name="psc", bufs=4, space=MemorySpace.PSUM) as psc,
                tc.tile_pool(name="outp", bufs=2) as outp,
            ):
                for ch in range(NRES):
                    for ct in range(2):
                        ot = outp.tile([P, CW], bf16, tag=f"ot{ct}",
                                       name=f"ot{ct}")
                        for s in range(CW // MMW):
                            sl = slice(s * MMW, (s + 1) * MMW)
                            ps = psc.tile([P, MMW], f32, tag="ops", name="ops")
                            for e in range(2):
                                nc.tensor.matmul(
                                    ps,
                                    mt_sb[e][:, ct * P:(ct + 1) * P],
                                    xn[ch][e][:, sl],
                                    start=(e == 0), stop=(e == 1),
                                )
                            # ot = rs*psum + r (per-partition scale+bias),
                            # alternating ACT/DVE
                            if nsb % 2 == 0:
                                nc.scalar.activation(
                                    out=ot[:, sl], in_=ps,
                                    func=mybir.ActivationFunctionType.Identity,
                                    bias=r_sb[ct], scale=rs_sb[ct])
                            else:
                                nc.vector.tensor_scalar(
                                    out=ot[:, sl], in0=ps,
                                    scalar1=rs_sb[ct], scalar2=r_sb[ct],
                                    op0=mybir.AluOpType.mult,
                                    op1=mybir.AluOpType.add)
                            nsb += 1
                            if ch == 0:
                                # first chunk: per-sub-block DMA so the
                                # output stream starts immediately
                                nc.sync.dma_start(
                                    out=out_ap[ct * P:(ct + 1) * P,
                                               ch * CW + s * MMW:
                                               ch * CW + (s + 1) * MMW],
                                    in_=ot[:, sl])
                        if ch > 0:
                            nc.sync.dma_start(
                                out=out_ap[ct * P:(ct + 1) * P,
                                           ch * CW:(ch + 1) * CW],
                                in_=ot)

    nc.compile()
    return nc


def _get_nc():
    key = (K_STREAM,)
    if key not in _BUILD_CACHE:
        _BUILD_CACHE[key] = _build(*key)
    return _BUILD_CACHE[key]


def kernel(x, wq, bq, wk, bk, wv, bv):
    global LAST_RESULT
    from concourse.bass_utils import run_bass_kernel_spmd

    nc = _get_nc()

    x = np.ascontiguousarray(np.asarray(x, dtype=np.float32))
    xf = x.reshape(B, C, N)
    wq = np.ascontiguousarray(np.asarray(wq, dtype=np.float32))
    wk = np.ascontiguousarray(np.asarray(wk, dtype=np.float32))
    wv = np.ascontiguousarray(np.asarray(wv, dtype=np.float32))
    bq = np.ascontiguousarray(np.asarray(bq, dtype=np.float32))
    bk = np.ascontiguousarray(np.asarray(bk, dtype=np.float32))
    bv = np.ascontiguousarray(np.asarray(bv, dtype=np.float32))

    in_maps = _make_in_maps(xf, wq, bq, wk, bk, wv, bv)

    res = run_bass_kernel_spmd(nc, in_maps, core_ids=list(range(NCORES)))
    LAST_RESULT = res

    out = np.empty((B, C, N), dtype=np.float32)
    for i in range(NCORES):
        b, h = i // 2, i % 2
        out[b, :, h * HALF:(h + 1) * HALF] = \
            res.results[i]["out"].astype(np.float32)
    return out.reshape(B, C, W, H)


def _make_in_maps(xf, wq, bq, wk, bk, wv, bv):
    bf16 = ml_dtypes.bfloat16
    ones_pad = np.zeros((N, 2), dtype=np.float32)
    ones_pad[:, 0] = 1.0
    wqt = np.ascontiguousarray(wq.T)
    wkt = np.ascontiguousarray(wk.T)
    in_maps = []
    for i in range(NCORES):
        b, h = i // 2, i % 2
        xTb = np.concatenate([xf[b].T, ones_pad], axis=1)
        # own spatial half first: the kernel keeps the own half resident in
        # natural layout and derives its output columns from it
        xt = np.concatenate([xTb[h * HALF:(h + 1) * HALF],
                             xTb[(1 - h) * HALF:(2 - h) * HALF]],
                            axis=0).astype(bf16)
        xn = np.ascontiguousarray(
            xf[b][:, h * HALF:(h + 1) * HALF]).astype(bf16)
        in_maps.append({
            "xt": xt, "xn": xn,
            "wqt": wqt, "wkt": wkt, "wv": wv,
            "bq": bq, "bk": bk, "bv": bv,
        })
    return in_maps


# ---------------------------------------------------------------------------
# Dev-loop helpers (not used by the grading path)
# ---------------------------------------------------------------------------

def timeline_ns():
    """Cost-model simulated duration of one core's program (ns)."""
    from concourse.timeline_sim import TimelineSim
    nc = _get_nc()
    ts = TimelineSim(nc)
    return ts.simulate()
